# revision 16
# baseline (speedup 1.0000x reference)
"""Trainium2 Bass kernel for nn_CLoss_68521908241007 (retrieval_knn).

Math (per the reference):
  sq_dist[i,j] = ||feat_i||^2 + ||feat2_j||^2 - 2 feat_i . feat2_j
  logits = -temp * sqrt(sq_dist)
  loss = mean_i( logsumexp_j(logits[i,:]) - logits[i, labels_i] )

Sharding: feat rows split across 8 cores (1024 queries each); feat2
replicated.  Each core returns S[p,b] = sum_j exp(-temp*dist) for query
q=b*128+p of its shard; the host finishes with ln(S) + temp*pdist and
the mean.

Per-core pipeline (PE assembles sq_dist; ACT only sqrts; DVE only exps):
  - PE fp8 DoubleRow matmul, contraction 256 = two planes:
      plane0: (-2*feat).T fp8e4  x  feat2.T fp8e4   -> -2 x.y
      plane1: ones rows 0..2     x  [yc; ym; yr]    -> +y_sq  (3-row exact
              e4m3 split of y_sq)
    One matmul per (qblock, 512-seg); PSUM gets sq_dist minus x_sq.
  - ACT: dist = Sqrt(psum + x_sq) straight from PSUM (bias = per-partition
    x_sq); 32 back-to-back sqrt calls, a single table load, no other ACT
    work -- ACT is the pacing engine at ~1.86us per [128,2048].
  - DVE: 16-bit Schraudolph exp:  w16 = int16(dist*(-temp*2^7/ln2) + B16);
    bits(w16) viewed as bf16 are exp(-temp*dist).  tensor_scalar runs in
    4x mode (16-bit in/out).  Row-sum = 3 bf16 tensor_add folds (2x mode)
    8192->1024 + one small 1x reduce.  ~7.5us/qblock, tracks the sqrt
    stream with no trailing ACT phase.
  - fp8 dot noise (~+-0.03 on dist), Schraudolph error (~+-4%/element),
    and bf16 fold noise are pseudo-random across 8192 keys and average
    out in S; end-to-end loss error ~1e-3 vs the 2e-2 gate (offline
    calibrated, B16 = 127*2^7 - 7.42 mean-zero constant).
"""

import numpy as np
from contextlib import ExitStack

import concourse.bass as bass
import concourse.bacc as bacc
import concourse.mybir as mybir
import concourse.tile as tile
from concourse.bass_utils import run_bass_kernel_spmd

AF = mybir.ActivationFunctionType
ALU = mybir.AluOpType
AX = mybir.AxisListType
f32 = mybir.dt.float32
bf16 = mybir.dt.bfloat16
i16 = mybir.dt.int16
fp8 = mybir.dt.float8e4

N_CORES = 8
N, M, D = 8192, 8192, 128
NQ = N // N_CORES        # queries per core
QB = NQ // 128           # q-blocks per core (8)
KSEG = 512               # keys per matmul
GRP = 4                  # k-segs per psum group (4 banks)
NGRP = (M // KSEG) // GRP  # 4 groups per q-block

A_EXP16 = 2.0 ** 7 / np.log(2.0)
B_EXP16 = float(127 * 2 ** 7) - 7.42
NCH = 16                 # rhs dma chunks (separate contiguous dram tensors)


def _body(tc, out_d, lhs0_d, lhsr_d, rhs_d, xb_d, aexp_d):
    nc = tc.nc
    with ExitStack() as ctx:
        singles = ctx.enter_context(tc.tile_pool(name="singles", bufs=1))
        distp = ctx.enter_context(tc.tile_pool(name="distp", bufs=3))
        foldp = ctx.enter_context(tc.tile_pool(name="foldp", bufs=2))
        psp = ctx.enter_context(tc.tile_pool(name="psp", bufs=2, space="PSUM"))

        # ---- warm the sqrt activation table immediately (the load is
        # ~1.3us and otherwise lands on the critical path of qb0)
        warm = singles.tile([128, 1], f32)
        nc.vector.memset(warm, 1.0)
        nc.scalar.activation(out=warm, in_=warm, func=AF.Sqrt,
                             bias=0.0, scale=1.0)

        # ---- inputs -> SBUF; qb0's lhs slice + rhs chunk 0 first so the
        # first matmul starts early (each its own contiguous dram tensor)
        rhs_sb = singles.tile([D, 2, M], fp8)
        w = M // NCH
        lhs_sb = singles.tile([D, 2, NQ], fp8)
        nc.sync.dma_start(out=lhs_sb[:, :, 0:128], in_=lhs0_d)
        nc.sync.dma_start(out=rhs_sb[:, :, 0:w], in_=rhs_d[0])
        for c in range(1, NCH):
            nc.sync.dma_start(out=rhs_sb[:, :, c * w:(c + 1) * w],
                              in_=rhs_d[c])
        nc.sync.dma_start(out=lhs_sb[:, :, 128:NQ], in_=lhsr_d)
        xb = singles.tile([128, QB], f32)
        nc.sync.dma_start(out=xb, in_=xb_d)
        aexp = singles.tile([128, 1], f32)
        nc.sync.dma_start(out=aexp, in_=aexp_d)

        S = singles.tile([128, QB], f32)
        w_t = singles.tile([128, M], i16)   # Schraudolph bits (1 buf)

        # ---- main stream: per qblock: 16 DoubleRow matmuls + 4 PSUM-read
        # sqrts; DVE exp+fold chain issues right behind each qblock.
        GW = GRP * KSEG  # 2048, one psum group's width
        for b in range(QB):
            dist_t = distp.tile([128, M], bf16, tag="dist")
            e_v = w_t.bitcast(bf16)
            for g in range(NGRP):
                ps = psp.tile([128, GW], f32, tag="ps")
                for si in range(GRP):
                    s = g * GRP + si
                    nc.tensor.matmul(
                        ps[:, si * KSEG:(si + 1) * KSEG],
                        lhsT=lhs_sb[:, :, b * 128:(b + 1) * 128],
                        rhs=rhs_sb[:, :, s * KSEG:(s + 1) * KSEG],
                        start=True, stop=True,
                        perf_mode=mybir.MatmulPerfMode.DoubleRow)
                nc.scalar.activation(
                    out=dist_t[:, g * GW:(g + 1) * GW],
                    in_=ps, func=AF.Sqrt, bias=xb[:, b:b + 1], scale=1.0)
                # Schraudolph exp (4x) per chunk, right behind the sqrt
                nc.vector.tensor_scalar(
                    out=w_t[:, g * GW:(g + 1) * GW],
                    in0=dist_t[:, g * GW:(g + 1) * GW],
                    scalar1=aexp[:, 0:1], scalar2=B_EXP16,
                    op0=ALU.mult, op1=ALU.add)
                # progressive pairwise bf16 folds: va after chunk 1,
                # vb after chunk 3, so only vc/v1/reduce trail the qblock
                if g == 1:
                    va = foldp.tile([128, GW], bf16, tag="va")
                    nc.vector.tensor_add(va, e_v[:, 0:GW], e_v[:, GW:2 * GW])
                elif g == 3:
                    vb = foldp.tile([128, GW], bf16, tag="vb")
                    nc.vector.tensor_add(
                        vb, e_v[:, 2 * GW:3 * GW], e_v[:, 3 * GW:])
            vc = foldp.tile([128, GW], bf16, tag="vc")
            nc.vector.tensor_add(vc, va, vb)
            v1 = foldp.tile([128, GW // 2], bf16, tag="v1")
            nc.vector.tensor_add(v1, vc[:, :GW // 2], vc[:, GW // 2:])
            nc.vector.tensor_reduce(
                out=S[:, b:b + 1], in_=v1, axis=AX.X, op=ALU.add)

        nc.sync.dma_start(out=out_d, in_=S)


def build_program():
    nc = bacc.Bacc("TRN2", target_bir_lowering=False, debug=False,
                   num_devices=N_CORES)
    lhs0 = nc.dram_tensor("lhs0", [D, 2, 128], fp8, kind="ExternalInput").ap()
    lhsr = nc.dram_tensor("lhsr", [D, 2, NQ - 128], fp8,
                          kind="ExternalInput").ap()
    rhs = [nc.dram_tensor(f"rhs{c}", [D, 2, M // NCH], fp8,
                          kind="ExternalInput").ap() for c in range(NCH)]
    xb = nc.dram_tensor("xb", [128, QB], f32, kind="ExternalInput").ap()
    aexp = nc.dram_tensor("aexp", [128, 1], f32, kind="ExternalInput").ap()
    out = nc.dram_tensor("out", [128, QB], f32, kind="ExternalOutput").ap()
    with tile.TileContext(nc) as tc:
        _body(tc, out, lhs0, lhsr, rhs, xb, aexp)
    nc.compile()
    return nc


def host_prep(feat, feat2, temp, labels):
    import ml_dtypes
    e4 = ml_dtypes.float8_e4m3
    feat = np.ascontiguousarray(np.asarray(feat, dtype=np.float32))
    feat2 = np.ascontiguousarray(np.asarray(feat2, dtype=np.float32))
    labels_np = np.asarray(labels).astype(np.int64)
    t = float(np.asarray(temp, dtype=np.float32))

    y_sq = np.einsum("md,md->m", feat2, feat2, dtype=np.float64)
    x_sq = np.einsum("nd,nd->n", feat, feat, dtype=np.float64)

    # rhs fp8 [D, 2, M]: plane0 = feat2.T, plane1 rows 0..2 = y_sq split;
    # shipped as NCH contiguous column-chunk tensors
    rhs = np.zeros((D, 2, M), dtype=e4)
    rhs[:, 0, :] = feat2.T.astype(e4)
    yc = np.floor(y_sq / 16.0) * 16.0
    ym = np.floor(y_sq - yc)
    yr = y_sq - yc - ym
    rhs[0, 1, :] = yc.astype(np.float32).astype(e4)
    rhs[1, 1, :] = ym.astype(np.float32).astype(e4)
    rhs[2, 1, :] = yr.astype(np.float32).astype(e4)
    w = M // NCH
    rhs_chunks = {f"rhs{c}": np.ascontiguousarray(rhs[:, :, c * w:(c + 1) * w])
                  for c in range(NCH)}

    aexp = np.full((128, 1), -t * A_EXP16, dtype=np.float32)

    diff = feat - feat2[labels_np]
    pdist = np.sqrt(np.einsum("nd,nd->n", diff, diff, dtype=np.float64))
    tpd = (t * pdist).astype(np.float64)          # [N], query order

    in_maps = []
    for c in range(N_CORES):
        fs = feat[c * NQ:(c + 1) * NQ]
        lhs = np.zeros((D, 2, NQ), dtype=e4)
        lhs[:, 0, :] = (-2.0 * fs.T).astype(e4)
        lhs[0:3, 1, :] = np.ones((3, NQ), dtype=e4)
        xbc = x_sq[c * NQ:(c + 1) * NQ].reshape(QB, 128).T
        in_maps.append({
            "lhs0": np.ascontiguousarray(lhs[:, :, 0:128]),
            "lhsr": np.ascontiguousarray(lhs[:, :, 128:NQ]),
            **rhs_chunks,
            "xb": np.ascontiguousarray(xbc).astype(np.float32),
            "aexp": aexp,
        })
    return in_maps, tpd


def finish(per_core_outs, tpd):
    # S[p, b] is sum_j exp(-t*dist) for query q = b*128 + p of that core
    srows = [np.asarray(o, dtype=np.float64).T.reshape(-1)
             for o in per_core_outs]
    S = np.concatenate(srows)                      # [N], query order
    loss = np.log(S) + tpd
    return np.float32(loss.mean())


_PROGRAM = None


def kernel(feat, feat2, temp, labels):
    global _PROGRAM
    if _PROGRAM is None:
        _PROGRAM = build_program()
    in_maps, tpd = host_prep(feat, feat2, temp, labels)
    res = run_bass_kernel_spmd(_PROGRAM, in_maps, core_ids=list(range(N_CORES)))
    return finish([r["out"] for r in res.results], tpd)


# revision 18
# speedup vs baseline: 1.0783x; 1.0783x over previous
"""Trainium2 Bass kernel for nn_CLoss_68521908241007 (retrieval_knn).

Math (per the reference):
  sq_dist[i,j] = ||feat_i||^2 + ||feat2_j||^2 - 2 feat_i . feat2_j
  logits = -temp * sqrt(sq_dist)
  loss = mean_i( logsumexp_j(logits[i,:]) - logits[i, labels_i] )

Sharding: feat rows split across 8 cores (1024 queries each); feat2
replicated.  Each core returns S[p,b] = sum_j exp(-temp*dist) for query
q=b*128+p of its shard; the host finishes with ln(S) + temp*pdist and
the mean.

Per-core pipeline (PE assembles sq_dist; ACT only sqrts; DVE only exps):
  - PE fp8 DoubleRow matmul, contraction 256 = two planes:
      plane0: (-2*feat).T fp8e4  x  feat2.T fp8e4   -> -2 x.y
      plane1: ones rows 0..2     x  [yc; ym; yr]    -> +y_sq  (3-row exact
              e4m3 split of y_sq)
    One matmul per (qblock, 512-seg); PSUM gets sq_dist minus x_sq.
  - ACT: dist = Sqrt(psum + x_sq) straight from PSUM (bias = per-partition
    x_sq); 32 back-to-back sqrt calls, a single table load, no other ACT
    work -- ACT is the pacing engine at ~1.86us per [128,2048].
  - DVE: 16-bit Schraudolph exp:  w16 = int16(dist*(-temp*2^7/ln2) + B16);
    bits(w16) viewed as bf16 are exp(-temp*dist).  tensor_scalar runs in
    4x mode (16-bit in/out).  Row-sum = 3 bf16 tensor_add folds (2x mode)
    8192->1024 + one small 1x reduce.  ~7.5us/qblock, tracks the sqrt
    stream with no trailing ACT phase.
  - fp8 dot noise (~+-0.03 on dist), Schraudolph error (~+-4%/element),
    and bf16 fold noise are pseudo-random across 8192 keys and average
    out in S; end-to-end loss error ~1e-3 vs the 2e-2 gate (offline
    calibrated, B16 = 127*2^7 - 7.42 mean-zero constant).
"""

import numpy as np
from contextlib import ExitStack

import concourse.bass as bass
import concourse.bacc as bacc
import concourse.mybir as mybir
import concourse.tile as tile
from concourse.bass_utils import run_bass_kernel_spmd

AF = mybir.ActivationFunctionType
ALU = mybir.AluOpType
AX = mybir.AxisListType
f32 = mybir.dt.float32
bf16 = mybir.dt.bfloat16
i16 = mybir.dt.int16
fp8 = mybir.dt.float8e4

N_CORES = 8
N, M, D = 8192, 8192, 128
NQ = N // N_CORES        # queries per core
QB = NQ // 128           # q-blocks per core (8)
KSEG = 512               # keys per matmul
GRP = 4                  # k-segs per psum group (4 banks)
NGRP = (M // KSEG) // GRP  # 4 groups per q-block

A_EXP16 = 2.0 ** 7 / np.log(2.0)
B_EXP16 = float(127 * 2 ** 7) - 7.42
NCH = 8                  # rhs dma chunks (separate contiguous dram tensors)


def _body(tc, out_d, lhs0_d, lhsr_d, rhs_d, xb_d, aexp_d):
    nc = tc.nc
    with ExitStack() as ctx:
        singles = ctx.enter_context(tc.tile_pool(name="singles", bufs=1))
        distp = ctx.enter_context(tc.tile_pool(name="distp", bufs=3))
        foldp = ctx.enter_context(tc.tile_pool(name="foldp", bufs=2))
        psp = ctx.enter_context(tc.tile_pool(name="psp", bufs=2, space="PSUM"))

        # ---- warm the sqrt activation table immediately (the load is
        # ~1.3us and otherwise lands on the critical path of qb0)
        warm = singles.tile([128, 1], f32)
        nc.vector.memset(warm, 1.0)
        nc.scalar.activation(out=warm, in_=warm, func=AF.Sqrt,
                             bias=0.0, scale=1.0)

        # ---- inputs -> SBUF; qb0's lhs slice + rhs chunk 0 first so the
        # first matmul starts early (each its own contiguous dram tensor)
        rhs_sb = singles.tile([D, 2, M], fp8)
        w = M // NCH
        lhs_sb = singles.tile([D, 2, NQ], fp8)
        xb = singles.tile([128, QB], f32)
        nc.sync.dma_start(out=xb, in_=xb_d)
        aexp = singles.tile([128, 1], f32)
        nc.sync.dma_start(out=aexp, in_=aexp_d)
        nc.sync.dma_start(out=lhs_sb[:, :, 0:128], in_=lhs0_d)
        nc.sync.dma_start(out=rhs_sb[:, :, 0:w], in_=rhs_d[0])
        for c in range(1, NCH):
            nc.sync.dma_start(out=rhs_sb[:, :, c * w:(c + 1) * w],
                              in_=rhs_d[c])
        nc.sync.dma_start(out=lhs_sb[:, :, 128:NQ], in_=lhsr_d)

        S = singles.tile([128, QB], f32)
        w_t = singles.tile([128, M], i16)   # Schraudolph bits (1 buf)

        # ---- main stream: per qblock: 16 DoubleRow matmuls + 4 PSUM-read
        # sqrts; DVE exp+fold chain issues right behind each qblock.
        GW = GRP * KSEG  # 2048, one psum group's width
        for b in range(QB):
            dist_t = distp.tile([128, M], bf16, tag="dist")
            e_v = w_t.bitcast(bf16)
            for g in range(NGRP):
                ps = psp.tile([128, GW], f32, tag="ps")
                for si in range(GRP):
                    s = g * GRP + si
                    nc.tensor.matmul(
                        ps[:, si * KSEG:(si + 1) * KSEG],
                        lhsT=lhs_sb[:, :, b * 128:(b + 1) * 128],
                        rhs=rhs_sb[:, :, s * KSEG:(s + 1) * KSEG],
                        start=True, stop=True,
                        perf_mode=mybir.MatmulPerfMode.DoubleRow)
                nc.scalar.activation(
                    out=dist_t[:, g * GW:(g + 1) * GW],
                    in_=ps, func=AF.Sqrt, bias=xb[:, b:b + 1], scale=1.0)
                # Schraudolph exp (4x) per chunk, right behind the sqrt
                nc.vector.tensor_scalar(
                    out=w_t[:, g * GW:(g + 1) * GW],
                    in0=dist_t[:, g * GW:(g + 1) * GW],
                    scalar1=aexp[:, 0:1], scalar2=B_EXP16,
                    op0=ALU.mult, op1=ALU.add)
                # progressive pairwise bf16 folds: va after chunk 1,
                # vb after chunk 3, so only vc/v1/reduce trail the qblock
                if g == 1:
                    va = foldp.tile([128, GW], bf16, tag="va")
                    nc.vector.tensor_add(va, e_v[:, 0:GW], e_v[:, GW:2 * GW])
                elif g == 3:
                    vb = foldp.tile([128, GW], bf16, tag="vb")
                    nc.vector.tensor_add(
                        vb, e_v[:, 2 * GW:3 * GW], e_v[:, 3 * GW:])
            vc = foldp.tile([128, GW], bf16, tag="vc")
            nc.vector.tensor_add(vc, va, vb)
            v1 = foldp.tile([128, GW // 2], bf16, tag="v1")
            nc.vector.tensor_add(v1, vc[:, :GW // 2], vc[:, GW // 2:])
            nc.vector.tensor_reduce(
                out=S[:, b:b + 1], in_=v1, axis=AX.X, op=ALU.add)

        nc.sync.dma_start(out=out_d, in_=S)


def build_program():
    nc = bacc.Bacc("TRN2", target_bir_lowering=False, debug=False,
                   num_devices=N_CORES)
    lhs0 = nc.dram_tensor("lhs0", [D, 2, 128], fp8, kind="ExternalInput").ap()
    lhsr = nc.dram_tensor("lhsr", [D, 2, NQ - 128], fp8,
                          kind="ExternalInput").ap()
    rhs = [nc.dram_tensor(f"rhs{c}", [D, 2, M // NCH], fp8,
                          kind="ExternalInput").ap() for c in range(NCH)]
    xb = nc.dram_tensor("xb", [128, QB], f32, kind="ExternalInput").ap()
    aexp = nc.dram_tensor("aexp", [128, 1], f32, kind="ExternalInput").ap()
    out = nc.dram_tensor("out", [128, QB], f32, kind="ExternalOutput").ap()
    with tile.TileContext(nc) as tc:
        _body(tc, out, lhs0, lhsr, rhs, xb, aexp)
    nc.compile()
    return nc


def host_prep(feat, feat2, temp, labels):
    import ml_dtypes
    e4 = ml_dtypes.float8_e4m3
    feat = np.ascontiguousarray(np.asarray(feat, dtype=np.float32))
    feat2 = np.ascontiguousarray(np.asarray(feat2, dtype=np.float32))
    labels_np = np.asarray(labels).astype(np.int64)
    t = float(np.asarray(temp, dtype=np.float32))

    y_sq = np.einsum("md,md->m", feat2, feat2, dtype=np.float64)
    x_sq = np.einsum("nd,nd->n", feat, feat, dtype=np.float64)

    # rhs fp8 [D, 2, M]: plane0 = feat2.T, plane1 rows 0..2 = y_sq split;
    # shipped as NCH contiguous column-chunk tensors
    rhs = np.zeros((D, 2, M), dtype=e4)
    rhs[:, 0, :] = feat2.T.astype(e4)
    yc = np.floor(y_sq / 16.0) * 16.0
    ym = np.floor(y_sq - yc)
    yr = y_sq - yc - ym
    rhs[0, 1, :] = yc.astype(np.float32).astype(e4)
    rhs[1, 1, :] = ym.astype(np.float32).astype(e4)
    rhs[2, 1, :] = yr.astype(np.float32).astype(e4)
    w = M // NCH
    rhs_chunks = {f"rhs{c}": np.ascontiguousarray(rhs[:, :, c * w:(c + 1) * w])
                  for c in range(NCH)}

    aexp = np.full((128, 1), -t * A_EXP16, dtype=np.float32)

    diff = feat - feat2[labels_np]
    pdist = np.sqrt(np.einsum("nd,nd->n", diff, diff, dtype=np.float64))
    tpd = (t * pdist).astype(np.float64)          # [N], query order

    in_maps = []
    for c in range(N_CORES):
        fs = feat[c * NQ:(c + 1) * NQ]
        lhs = np.zeros((D, 2, NQ), dtype=e4)
        lhs[:, 0, :] = (-2.0 * fs.T).astype(e4)
        lhs[0:3, 1, :] = np.ones((3, NQ), dtype=e4)
        xbc = x_sq[c * NQ:(c + 1) * NQ].reshape(QB, 128).T
        in_maps.append({
            "lhs0": np.ascontiguousarray(lhs[:, :, 0:128]),
            "lhsr": np.ascontiguousarray(lhs[:, :, 128:NQ]),
            **rhs_chunks,
            "xb": np.ascontiguousarray(xbc).astype(np.float32),
            "aexp": aexp,
        })
    return in_maps, tpd


def finish(per_core_outs, tpd):
    # S[p, b] is sum_j exp(-t*dist) for query q = b*128 + p of that core
    srows = [np.asarray(o, dtype=np.float64).T.reshape(-1)
             for o in per_core_outs]
    S = np.concatenate(srows)                      # [N], query order
    loss = np.log(S) + tpd
    return np.float32(loss.mean())


_PROGRAM = None


def kernel(feat, feat2, temp, labels):
    global _PROGRAM
    if _PROGRAM is None:
        _PROGRAM = build_program()
    in_maps, tpd = host_prep(feat, feat2, temp, labels)
    res = run_bass_kernel_spmd(_PROGRAM, in_maps, core_ids=list(range(N_CORES)))
    return finish([r["out"] for r in res.results], tpd)
